# revision 23
# baseline (speedup 1.0000x reference)
"""Trainium2 Bass kernel for a 2-layer GAT (PyG-style) + linear head.

Strategy (8 NeuronCores, SPMD):
  - Nodes are partitioned contiguously across cores (6250 per core); each core
    owns the destination segments of its nodes.  Edges (incl. self-loops) are
    routed to the core owning their destination, sorted by destination, and
    grouped into 128-destination blocks.
  - Layer tables (per-node features needed by gathers) are computed
    distributed, then AllGathered so every core can gather any source row.
  - Per 128-edge tile: dma_gather of source rows, per-edge attention
    p = exp(leaky_relu(a_src[src]+a_dst[dst])) , and a 0/1 indicator matrix
    I[k, d] = (dst_rel[k] == d) used as the stationary matmul operand to
    scatter-accumulate both the weighted messages and the softmax denominators
    into PSUM over the block.
  - dma_gather indices are int16, so each block's edges are split into a
    "lo" group (src < 32768, gathered at table base 0) and a "hi" group
    (src >= 32768, gathered at base N-32768); one table, two base offsets.

Host-side work is limited to integer index manipulation / padding / layout
(replicating weights, transposing x) — all floating-point math runs on device.
"""

import hashlib
import math
import sys

for _p in ("/opt/trn_rl_repo",):
    if _p not in sys.path:
        sys.path.insert(0, _p)

import copy

import numpy as np

import concourse.mybir as mybir
from concourse import bass
from concourse.library_config import mlp as _mlp_lib
from concourse.library_overlay import lower_extended_insts
from concourse.tile import TileContext

F32 = mybir.dt.float32
BF16 = mybir.dt.bfloat16
I16 = mybir.dt.int16
P = 128
GCAP = 7  # max 128-row tiles per dma_gather (SWDGE ring < 1024 descs)
AX = mybir.AxisListType.X
OP = mybir.AluOpType
ACT = mybir.ActivationFunctionType

FULL_CFG = dict(N=50000, E=800000, IN=256, H=8, C1=64, C2=32, OUT=40, NC=8,
                NEG=0.2)


def _pad256_elems(n_elems, esize):
    b = n_elems * esize
    return ((b + 255) // 256 * 256) // esize


class Cfg:
    def __init__(self, **kw):
        self.__dict__.update(kw)
        self.N_OWN = self.N // self.NC
        assert self.N_OWN * self.NC == self.N
        self.HC1 = self.H * self.C1
        self.NBLK = (self.N_OWN + P - 1) // P
        # single rank-major table [N]; int16 gather indices are handled by
        # two base offsets: src < LOCUT uses base 0, src >= LOCUT uses
        # base HIBASE (so idx-HIBASE <= 32767).
        self.LOCUT = 32768 if self.N > 32768 else (self.N + 1) // 2
        self.HIBASE = max(0, self.N - 32768)
        assert self.LOCUT - self.HIBASE <= 32768
        # layer-1 table row: HC1 bf16 + H f32 (as 2*H bf16), padded to 256B
        self.ROW1 = _pad256_elems(self.HC1 + 2 * self.H, 2)
        # layer-2 table row: C2+1 f32 padded to 256B
        self.ROW2 = _pad256_elems(self.C2 + 1, 4)
        # a_dst tables: H (or 1) f32 padded to 256B
        self.ROWT = _pad256_elems(max(self.H, 1), 4)


# ---------------------------------------------------------------- host side


def _pack_idx16(vals):
    """dma_gather int16 index layout: [128, len/16]; idx j at [j%16, j//16],
    replicated down the partition axis for the 8 Q7 cores."""
    vals = np.asarray(vals, np.int16)
    assert len(vals) % 16 == 0
    arr = vals.reshape(-1, 16).T  # [16, S]
    return np.ascontiguousarray(np.tile(arr, (8, 1)))  # [128, S]


def preprocess(edge_index, cfg):
    """Shard + sort edges, build per-core padded gather structures."""
    c = cfg
    ei = np.asarray(edge_index).astype(np.int64)
    loop = np.arange(c.N, dtype=np.int64)
    src_all = np.concatenate([ei[0], loop])
    dst_all = np.concatenate([ei[1], loop])

    per_core_raw = []
    cnt = np.zeros((c.NC, c.NBLK, 2), np.int64)
    for r in range(c.NC):
        sel = dst_all // c.N_OWN == r
        s = src_all[sel]
        d = dst_all[sel] - r * c.N_OWN
        blk = d >> 7
        hi = (s >= c.LOCUT).astype(np.int64)
        order = np.lexsort((d, hi, blk))
        s, d, blk, hi = s[order], d[order], blk[order], hi[order]
        np.add.at(cnt[r], (blk, hi), 1)
        per_core_raw.append((s, d, blk, hi))

    T_lo = ((cnt[:, :, 0] + P - 1) // P).max(axis=0)  # [NBLK]
    T_hi = ((cnt[:, :, 1] + P - 1) // P).max(axis=0)
    T_all = T_lo + T_hi
    n_tiles = int(T_all.sum())

    cores = []
    for r in range(c.NC):
        s, d, blk, hi = per_core_raw[r]
        lo_idx = np.zeros(int(T_lo.sum()) * P, np.int16)
        hi_idx = np.zeros(int(T_hi.sum()) * P, np.int16)
        t_idx = np.zeros(n_tiles * P, np.int16)
        drel = np.full(n_tiles * P, 999.0, np.float32)

        lo_off = 0
        hi_off = 0
        e_off = 0
        pos = 0  # cursor into sorted edge arrays
        for b in range(c.NBLK):
            for which, T_b, arr, off in ((0, int(T_lo[b]), lo_idx, lo_off),
                                         (1, int(T_hi[b]), hi_idx, hi_off)):
                n = int(cnt[r, b, which])
                sb = s[pos : pos + n]
                db = d[pos : pos + n]
                pos += n
                tpos = sb if which == 0 else sb - c.HIBASE
                arr[off : off + n] = tpos.astype(np.int16)
                # t-gather (local dst) + dst_rel share the block edge order
                t_idx[e_off : e_off + n] = db.astype(np.int16)
                drel[e_off : e_off + n] = (db - b * P).astype(np.float32)
                if which == 0:
                    lo_off += T_b * P
                else:
                    hi_off += T_b * P
                e_off += T_b * P
        assert pos == len(s)

        cores.append(dict(
            lo_idx=_pack_idx16(lo_idx),
            hi_idx=_pack_idx16(hi_idx),
            t_idx=_pack_idx16(t_idx),
            dst_rel=np.ascontiguousarray(
                drel.reshape(n_tiles, P).T).astype(np.float32),
        ))

    return dict(T_lo=T_lo.astype(int), T_hi=T_hi.astype(int), cores=cores)


# ---------------------------------------------------------------- program


def _fix_multiwait(nc):
    """Walrus in this toolchain rejects >1 sync wait on most instruction
    encodings.  Hoist extra waits onto pure EventSemaphore instructions just
    before the affected instruction (same engine, sequencer-ordered)."""
    def fragile(inst):
        return True

    for fn in nc.m.functions:
        for bb in fn.blocks:
            insts = bb.instructions
            i = 0
            while i < len(insts):
                inst = insts[i]
                si = inst.sync_info
                if (fragile(inst) and si is not None and si.on_wait
                        and len(si.on_wait) > 1):
                    waits = list(si.on_wait)
                    new = []
                    for k, w in enumerate(waits[:-1]):
                        d = mybir.InstEventSemaphore(
                            name=f"{inst.name}_hw{k}", engine=inst.engine,
                            ins=[], outs=[])
                        d.sync_info = mybir.SyncInfo(on_wait=[w], on_update=[])
                        new.append(d)
                        nc.register_instruction(d)
                    si.on_wait = [waits[-1]]
                    insts[i:i] = new
                    i += len(new)
                i += 1


def build_program(cfg, T_lo, T_hi, skip=()):
    c = cfg
    T_all = [int(a + b) for a, b in zip(T_lo, T_hi)]
    n_tiles = int(sum(T_all))
    S_lo = int(sum(T_lo)) * 8
    S_hi = int(sum(T_hi)) * 8
    S_t = n_tiles * 8
    T_LO_MAX = int(max(T_lo))
    T_HI_MAX = int(max(T_hi))
    T_ALL_MAX = int(max(T_all))
    IN_CH = c.IN // P          # contraction chunks for layer-1 matmul
    HC_CH = c.HC1 // P if c.HC1 >= P else 0  # chunks for layer-2 matmul

    nc = bass.Bass(trn_type="TRN2", num_devices=c.NC)

    # ---- I/O
    xT = nc.dram_tensor("xT", [c.IN, c.N_OWN], F32, kind="ExternalInput")
    W1 = nc.dram_tensor("W1", [c.IN, c.HC1], F32, kind="ExternalInput")
    at1s = nc.dram_tensor("at1s", [P, c.HC1], F32, kind="ExternalInput")
    at1d = nc.dram_tensor("at1d", [P, c.HC1], F32, kind="ExternalInput")
    b1b = nc.dram_tensor("b1b", [P, c.HC1], F32, kind="ExternalInput")
    W2 = nc.dram_tensor("W2", [c.HC1, c.C2], F32, kind="ExternalInput")
    at2s = nc.dram_tensor("at2s", [P, c.C2], F32, kind="ExternalInput")
    at2d = nc.dram_tensor("at2d", [P, c.C2], F32, kind="ExternalInput")
    b2b = nc.dram_tensor("b2b", [P, c.C2], F32, kind="ExternalInput")
    fcW = nc.dram_tensor("fcW", [c.C2, c.OUT], F32, kind="ExternalInput")
    fcbb = nc.dram_tensor("fcbb", [P, c.OUT], F32, kind="ExternalInput")
    iota = nc.dram_tensor("iota", [P, P], F32, kind="ExternalInput")
    ident = nc.dram_tensor("ident", [P, P], F32, kind="ExternalInput")
    lo_idx = nc.dram_tensor("lo_idx", [P, S_lo], I16, kind="ExternalInput")
    hi_idx = nc.dram_tensor("hi_idx", [P, S_hi], I16, kind="ExternalInput")
    t_idx = nc.dram_tensor("t_idx", [P, S_t], I16, kind="ExternalInput")
    dst_rel = nc.dram_tensor("dst_rel", [P, n_tiles], F32, kind="ExternalInput")
    emb = nc.dram_tensor("emb", [c.N_OWN, c.C2], F32, kind="ExternalOutput")
    logits = nc.dram_tensor("logits", [c.N_OWN, c.OUT], F32,
                            kind="ExternalOutput")

    # ---- internal DRAM
    F1_own = nc.dram_tensor("F1_own", [c.N_OWN, c.ROW1], BF16,
                            kind="Internal")
    F1_full = nc.dram_tensor("F1_full", [c.N, c.ROW1], BF16, kind="Internal",
                             addr_space="Shared")
    adst1 = nc.dram_tensor("adst1", [c.N_OWN, c.ROWT], F32, kind="Internal")
    F2_own = nc.dram_tensor("F2_own", [c.N_OWN, c.ROW2], F32,
                            kind="Internal")
    F2_full = nc.dram_tensor("F2_full", [c.N, c.ROW2], F32, kind="Internal",
                             addr_space="Shared")
    adst2 = nc.dram_tensor("adst2", [c.N_OWN, c.ROWT], F32, kind="Internal")

    rg = [list(range(c.NC))]

    with TileContext(nc) as tc:
        with (
            tc.tile_pool(name="cn", bufs=1) as cn,
            tc.tile_pool(name="sb", bufs=getattr(c, "SB_BUFS", 2)) as sb,
            tc.tile_pool(name="sb3", bufs=getattr(c, "SB3_BUFS", 8)) as sb3,
            tc.tile_pool(name="ps", bufs=2, space="PSUM") as ps,
        ):
            nc.gpsimd.load_library(_mlp_lib)

            # one shared register per distinct gather size (to_reg would
            # otherwise allocate a fresh Pool register per gather call)
            nidx_reg = {}
            for t in range(1, GCAP + 1):
                v = t * P
                r = nc.gpsimd.alloc_register(f"nidxv{v}")
                nc.gpsimd.reg_mov(r, v)
                nidx_reg[v] = r

            # SWDGE descriptor ring holds <1024 descriptors; split gathers
            # into <=GCAP-tile chunks and rotate the 4 SWDGE queues.
            gq = [0]

            def gather(out_tile, row, table, idx_sb, col0, T):
                t0 = 0
                while t0 < T:
                    tcnt = min(GCAP, T - t0)
                    q = 0
                    nc.gpsimd.dma_gather(
                        out_ap=out_tile[:, t0 * row:(t0 + tcnt) * row]
                        .rearrange("p (t e) -> p t e", e=row),
                        in_ap=table,
                        idxs_ap=idx_sb[:, col0 + t0 * 8:
                                       col0 + (t0 + tcnt) * 8],
                        num_idxs=tcnt * P, num_idxs_reg=nidx_reg[tcnt * P],
                        elem_size=row, queue_num=q)
                    t0 += tcnt

            # ---- constants to SBUF
            def cload(t, shape, dtype=F32, tag=None):
                tl = cn.tile(shape, dtype, tag=tag or t.name)
                nc.sync.dma_start(out=tl[:], in_=t[:])
                return tl

            iota_sb = cload(iota, [P, P])
            ident_sb = cload(ident, [P, P])
            at1s_sb = cload(at1s, [P, c.HC1])
            at1d_sb = cload(at1d, [P, c.HC1])
            b1_sb = cload(b1b, [P, c.HC1])
            at2s_sb = cload(at2s, [P, c.C2])
            at2d_sb = cload(at2d, [P, c.C2])
            b2_sb = cload(b2b, [P, c.C2])
            fcb_sb = cload(fcbb, [P, c.OUT])
            fcW_sb = cn.tile([c.C2, c.OUT], F32, tag="fcW")
            nc.sync.dma_start(out=fcW_sb[:], in_=fcW[:])
            W1_sb = [cn.tile([P, c.HC1], F32, name=f"W1c{k}", tag=f"W1_{k}")
                     for k in range(IN_CH)]
            for k in range(IN_CH):
                nc.sync.dma_start(out=W1_sb[k][:], in_=W1[k * P:(k + 1) * P, :])
            n_w2 = max(HC_CH, 1)
            w2p = c.HC1 // n_w2
            W2_sb = [cn.tile([w2p, c.C2], F32, name=f"W2c{k}", tag=f"W2_{k}")
                     for k in range(n_w2)]
            for k in range(n_w2):
                nc.sync.dma_start(out=W2_sb[k][:], in_=W2[k * w2p:(k + 1) * w2p, :])
            loidx_sb = cload(lo_idx, [P, S_lo], I16)
            hiidx_sb = cload(hi_idx, [P, S_hi], I16)
            tidx_sb = cload(t_idx, [P, S_t], I16)
            drel_sb = cload(dst_rel, [P, n_tiles])

            # ===== Phase A: xl1/a_src/a_dst for own nodes -> F1_own,
            # then one AllGather to the full table.
            for nt in range(c.NBLK):
                nn = min(P, c.N_OWN - nt * P)
                pxl = ps.tile([P, c.HC1], F32, tag="pmsg", bufs=getattr(c, 'PMSG_BUFS', 2))
                for k in range(IN_CH):
                    xt = sb.tile([P, P], F32, tag="xt")
                    nc.sync.dma_start(
                        out=xt[:, :nn], in_=xT[k * P:(k + 1) * P,
                                               nt * P: nt * P + nn])
                    nc.tensor.matmul(pxl[:nn, :], lhsT=xt[:, :nn],
                                     rhs=W1_sb[k][:],
                                     start=(k == 0), stop=(k == IN_CH - 1))
                row = sb.tile([P, c.ROW1], BF16, tag="f1row")
                if c.ROW1 > c.HC1 + 2 * c.H:
                    nc.vector.memset(row[:, c.HC1 + 2 * c.H:], 0.0)
                nc.vector.tensor_copy(out=row[:nn, 0:c.HC1], in_=pxl[:nn, :])
                tmp = sb.tile([P, c.HC1], F32, tag="atmp")
                asrc = sb.tile([P, c.H], F32, tag="asrc")
                adst_row = sb.tile([P, c.ROWT], F32, tag="adrow")
                if c.ROWT > c.H:
                    nc.vector.memset(adst_row[:, c.H:], 0.0)
                nc.vector.tensor_tensor(out=tmp[:nn, :], in0=pxl[:nn, :],
                                        in1=at1s_sb[:nn, :], op=OP.mult)
                nc.vector.reduce_sum(
                    out=asrc[:nn, :],
                    in_=tmp[:nn, :].rearrange("p (h c) -> p h c", c=c.C1),
                    axis=AX)
                nc.vector.tensor_tensor(out=tmp[:nn, :], in0=pxl[:nn, :],
                                        in1=at1d_sb[:nn, :], op=OP.mult)
                nc.vector.reduce_sum(
                    out=adst_row[:nn, 0:c.H],
                    in_=tmp[:nn, :].rearrange("p (h c) -> p h c", c=c.C1),
                    axis=AX)
                nc.vector.tensor_copy(
                    out=row[:nn, c.HC1:c.HC1 + 2 * c.H].bitcast(F32),
                    in_=asrc[:nn, :])
                nc.sync.dma_start(out=F1_own[nt * P: nt * P + nn, :],
                                  in_=row[:nn, :])
                nc.sync.dma_start(out=adst1[nt * P: nt * P + nn, :],
                                  in_=adst_row[:nn, :])

            if "ag1" not in skip:
                nc.gpsimd.collective_compute(
                    "AllGather", OP.bypass, ins=[F1_own[:]],
                    outs=[F1_full[:]], replica_groups=rg)

            # ================= Phase B: layer-1 message passing per block
            lo_col = 0
            hi_col = 0
            t_col = 0
            tile_col = 0
            for b in range(c.NBLK if "phaseb" not in skip else 0):
                Tl, Th, Ta = int(T_lo[b]), int(T_hi[b]), T_all[b]
                nb = min(P, c.N_OWN - b * P)
                glo = ghi = None
                if Tl:
                    glo = sb.tile([P, T_LO_MAX * c.ROW1], BF16, tag="glo", bufs=getattr(c, "GBUFS", 3))
                    gather(glo, c.ROW1, F1_full[0:, :], loidx_sb, lo_col, Tl)
                    lo_col += Tl * 8
                if Th:
                    ghi = sb.tile([P, T_HI_MAX * c.ROW1], BF16, tag="ghi", bufs=getattr(c, "GBUFS", 3))
                    gather(ghi, c.ROW1, F1_full[c.HIBASE:, :], hiidx_sb, hi_col, Th)
                    hi_col += Th * 8
                tg = sb.tile([P, T_ALL_MAX * c.ROWT], F32, tag="tg", bufs=getattr(c, "GBUFS", 3))
                gather(tg, c.ROWT, adst1[:], tidx_sb, t_col, Ta)
                t_col += Ta * 8

                # scalar phase: s, t -> p (f32), p_bf
                HT = c.H
                sf = sb.tile([P, T_ALL_MAX * HT], F32, tag="sf")
                if Tl:
                    nc.vector.tensor_copy(
                        out=sf[:, :Tl * HT].rearrange("p (t h) -> p t h", h=HT),
                        in_=glo[:, :Tl * c.ROW1].rearrange(
                            "p (t e) -> p t e", e=c.ROW1)[
                            :, :, c.HC1:c.HC1 + 2 * HT].bitcast(F32))
                if Th:
                    nc.vector.tensor_copy(
                        out=sf[:, Tl * HT: Ta * HT].rearrange(
                            "p (t h) -> p t h", h=HT),
                        in_=ghi[:, :Th * c.ROW1].rearrange(
                            "p (t e) -> p t e", e=c.ROW1)[
                            :, :, c.HC1:c.HC1 + 2 * HT].bitcast(F32))
                ef = sb.tile([P, T_ALL_MAX * HT], F32, tag="ef")
                nc.vector.tensor_tensor(
                    out=ef[:, :Ta * HT].rearrange("p (t h) -> p t h", h=HT),
                    in0=sf[:, :Ta * HT].rearrange("p (t h) -> p t h", h=HT),
                    in1=tg[:, :Ta * c.ROWT].rearrange(
                        "p (t e) -> p t e", e=c.ROWT)[:, :, 0:HT],
                    op=OP.add)
                e2 = sb.tile([P, T_ALL_MAX * HT], F32, tag="e2")
                nc.vector.tensor_scalar_mul(e2[:, :Ta * HT], ef[:, :Ta * HT],
                                            c.NEG)
                nc.vector.tensor_tensor(out=ef[:, :Ta * HT],
                                        in0=ef[:, :Ta * HT],
                                        in1=e2[:, :Ta * HT], op=OP.max)
                pf = sb.tile([P, T_ALL_MAX * HT], F32, tag="pf")
                nc.scalar.activation(pf[:, :Ta * HT], ef[:, :Ta * HT], ACT.Exp)
                pbf = sb.tile([P, T_ALL_MAX * HT], BF16, tag="pbf")
                nc.vector.tensor_copy(out=pbf[:, :Ta * HT], in_=pf[:, :Ta * HT])

                pmsg = ps.tile([P, c.HC1], F32, tag="pmsg", bufs=getattr(c, 'PMSG_BUFS', 2))
                pden = ps.tile([P, c.H], F32, tag="pden", bufs=getattr(c, 'PDEN_BUFS', 2))
                for i in range(Ta):
                    ind = sb3.tile([P, P], BF16, tag="ind")
                    nc.vector.tensor_scalar(
                        out=ind[:], in0=iota_sb[:],
                        scalar1=drel_sb[:, tile_col + i: tile_col + i + 1],
                        scalar2=None, op0=OP.is_equal)
                    if i < Tl:
                        gsrc = glo[:, :Tl * c.ROW1].rearrange(
                            "p (t e) -> p t e", e=c.ROW1)[:, i, 0:c.HC1]
                    else:
                        gsrc = ghi[:, :Th * c.ROW1].rearrange(
                            "p (t e) -> p t e", e=c.ROW1)[:, i - Tl, 0:c.HC1]
                    msg = sb3.tile([P, c.HC1], BF16, tag="msg")
                    nc.vector.tensor_tensor(
                        out=msg[:].rearrange("p (h c) -> p h c", c=c.C1),
                        in0=gsrc.rearrange("p (h c) -> p h c", c=c.C1),
                        in1=pbf[:, i * HT:(i + 1) * HT].to_broadcast(
                            [P, HT, c.C1]),
                        op=OP.mult)
                    nc.tensor.matmul(pmsg[:], lhsT=ind[:], rhs=msg[:],
                                     start=(i == 0), stop=(i == Ta - 1),
                                     skip_group_check=True)
                    nc.tensor.matmul(pden[:], lhsT=ind[:],
                                     rhs=pbf[:, i * HT:(i + 1) * HT],
                                     start=(i == 0), stop=(i == Ta - 1),
                                     skip_group_check=True)
                tile_col += Ta

                # epilogue: h1 = relu(pmsg/denom + b1)
                dens = sb.tile([P, c.H], F32, tag="dens")
                nc.vector.tensor_scalar(out=dens[:], in0=pden[:],
                                        scalar1=1e-16, scalar2=None,
                                        op0=OP.add)
                rden = sb.tile([P, c.H], F32, tag="rden")
                nc.vector.reciprocal(rden[:], dens[:])
                h1 = sb.tile([P, c.HC1], F32, tag="h1")
                nc.vector.tensor_tensor(
                    out=h1[:].rearrange("p (h c) -> p h c", c=c.C1),
                    in0=pmsg[:].rearrange("p (h c) -> p h c", c=c.C1),
                    in1=rden[:].to_broadcast([P, c.H, c.C1]), op=OP.mult)
                nc.vector.tensor_tensor(out=h1[:], in0=h1[:], in1=b1_sb[:],
                                        op=OP.add)
                nc.scalar.activation(h1[:], h1[:], ACT.Relu)

                # layer-2 prep: xl2 = h1 @ W2 (transpose h1 chunks on PE)
                pxl2 = ps.tile([P, c.C2], F32, tag="paux", bufs=getattr(c, 'PAUX_BUFS', 2))
                for k in range(n_w2):
                    ptr = ps.tile([P, P], F32, tag="ptr", bufs=getattr(c, 'PTR_BUFS', 2))
                    nc.tensor.transpose(ptr[:w2p, :],
                                        h1[:, k * w2p:(k + 1) * w2p],
                                        ident_sb[:])
                    h1t = sb.tile([P, P], F32, tag="h1t")
                    nc.vector.tensor_copy(out=h1t[:w2p, :], in_=ptr[:w2p, :])
                    nc.tensor.matmul(pxl2[:], lhsT=h1t[:w2p, :],
                                     rhs=W2_sb[k][:], start=(k == 0),
                                     stop=(k == n_w2 - 1))
                xl2 = sb.tile([P, c.C2], F32, tag="xl2")
                nc.vector.tensor_copy(out=xl2[:], in_=pxl2[:])
                row2 = sb.tile([P, c.ROW2], F32, tag="row2")
                if c.ROW2 > c.C2 + 1:
                    nc.vector.memset(row2[:, c.C2 + 1:], 0.0)
                nc.vector.tensor_copy(out=row2[:, 0:c.C2], in_=xl2[:])
                t2 = sb.tile([P, c.C2], F32, tag="t2tmp")
                nc.vector.tensor_tensor(out=t2[:], in0=xl2[:], in1=at2s_sb[:],
                                        op=OP.mult)
                nc.vector.reduce_sum(
                    out=row2[:, c.C2:c.C2 + 1],
                    in_=t2[:].rearrange("p (o c) -> p o c", o=1), axis=AX)
                ad2 = sb.tile([P, c.ROWT], F32, tag="ad2")
                if c.ROWT > 1:
                    nc.vector.memset(ad2[:, 1:], 0.0)
                nc.vector.tensor_tensor(out=t2[:], in0=xl2[:], in1=at2d_sb[:],
                                        op=OP.mult)
                nc.vector.reduce_sum(
                    out=ad2[:, 0:1],
                    in_=t2[:].rearrange("p (o c) -> p o c", o=1), axis=AX)
                nc.sync.dma_start(out=F2_own[b * P: b * P + nb, :],
                                  in_=row2[:nb, :])
                nc.sync.dma_start(out=adst2[b * P: b * P + nb, :],
                                  in_=ad2[:nb, :])

            if "ag2" not in skip:
                nc.gpsimd.collective_compute(
                    "AllGather", OP.bypass, ins=[F2_own[:]],
                    outs=[F2_full[:]], replica_groups=rg)

            # ================= Phase C: layer-2 message passing + head
            lo_col = 0
            hi_col = 0
            t_col = 0
            tile_col = 0
            for b in range(c.NBLK if "phasec" not in skip else 0):
                Tl, Th, Ta = int(T_lo[b]), int(T_hi[b]), T_all[b]
                nb = min(P, c.N_OWN - b * P)
                g2lo = g2hi = None
                if Tl:
                    g2lo = sb.tile([P, T_LO_MAX * c.ROW2], F32, tag="glo", bufs=getattr(c, "GBUFS", 3))
                    gather(g2lo, c.ROW2, F2_full[0:, :], loidx_sb, lo_col, Tl)
                    lo_col += Tl * 8
                if Th:
                    g2hi = sb.tile([P, T_HI_MAX * c.ROW2], F32, tag="ghi", bufs=getattr(c, "GBUFS", 3))
                    gather(g2hi, c.ROW2, F2_full[c.HIBASE:, :], hiidx_sb, hi_col, Th)
                    hi_col += Th * 8
                tg2 = sb.tile([P, T_ALL_MAX * c.ROWT], F32, tag="tg", bufs=getattr(c, "GBUFS", 3))
                gather(tg2, c.ROWT, adst2[:], tidx_sb, t_col, Ta)
                t_col += Ta * 8

                sf2 = sb.tile([P, T_ALL_MAX], F32, tag="sf2")
                if Tl:
                    nc.vector.tensor_copy(
                        out=sf2[:, :Tl].rearrange("p (t o) -> p t o", o=1),
                        in_=g2lo[:, :Tl * c.ROW2].rearrange(
                            "p (t e) -> p t e", e=c.ROW2)[
                            :, :, c.C2:c.C2 + 1])
                if Th:
                    nc.vector.tensor_copy(
                        out=sf2[:, Tl:Ta].rearrange("p (t o) -> p t o", o=1),
                        in_=g2hi[:, :Th * c.ROW2].rearrange(
                            "p (t e) -> p t e", e=c.ROW2)[
                            :, :, c.C2:c.C2 + 1])
                ef2 = sb.tile([P, T_ALL_MAX], F32, tag="ef2")
                nc.vector.tensor_tensor(
                    out=ef2[:, :Ta].rearrange("p (t o) -> p t o", o=1),
                    in0=sf2[:, :Ta].rearrange("p (t o) -> p t o", o=1),
                    in1=tg2[:, :Ta * c.ROWT].rearrange(
                        "p (t e) -> p t e", e=c.ROWT)[:, :, 0:1],
                    op=OP.add)
                e22 = sb.tile([P, T_ALL_MAX], F32, tag="e22")
                nc.vector.tensor_scalar_mul(e22[:, :Ta], ef2[:, :Ta], c.NEG)
                nc.vector.tensor_tensor(out=ef2[:, :Ta], in0=ef2[:, :Ta],
                                        in1=e22[:, :Ta], op=OP.max)
                pf2 = sb.tile([P, T_ALL_MAX], F32, tag="pf2")
                nc.scalar.activation(pf2[:, :Ta], ef2[:, :Ta], ACT.Exp)

                pout2 = ps.tile([P, c.C2 + 1], F32, tag="pden", bufs=getattr(c, 'PDEN_BUFS', 2))
                for i in range(Ta):
                    ind2 = sb3.tile([P, P], F32, tag="ind2")
                    nc.vector.tensor_scalar(
                        out=ind2[:], in0=iota_sb[:],
                        scalar1=drel_sb[:, tile_col + i: tile_col + i + 1],
                        scalar2=None, op0=OP.is_equal)
                    if i < Tl:
                        gsrc2 = g2lo[:, :Tl * c.ROW2].rearrange(
                            "p (t e) -> p t e", e=c.ROW2)[:, i, 0:c.C2]
                    else:
                        gsrc2 = g2hi[:, :Th * c.ROW2].rearrange(
                            "p (t e) -> p t e", e=c.ROW2)[:, i - Tl, 0:c.C2]
                    rhs2 = sb3.tile([P, c.C2 + 1], F32, tag="rhs2")
                    nc.vector.tensor_scalar(
                        out=rhs2[:, 0:c.C2], in0=gsrc2,
                        scalar1=pf2[:, i:i + 1], scalar2=None, op0=OP.mult)
                    nc.vector.tensor_copy(out=rhs2[:, c.C2:c.C2 + 1],
                                          in_=pf2[:, i:i + 1])
                    nc.tensor.matmul(pout2[:], lhsT=ind2[:], rhs=rhs2[:],
                                     start=(i == 0), stop=(i == Ta - 1))
                tile_col += Ta

                dens2 = sb.tile([P, 1], F32, tag="dens2")
                nc.vector.tensor_scalar(out=dens2[:], in0=pout2[:, c.C2:],
                                        scalar1=1e-16, scalar2=None,
                                        op0=OP.add)
                rden2 = sb.tile([P, 1], F32, tag="rden2")
                nc.vector.reciprocal(rden2[:], dens2[:])
                h2 = sb.tile([P, c.C2], F32, tag="h2")
                nc.vector.tensor_scalar(out=h2[:], in0=pout2[:, 0:c.C2],
                                        scalar1=rden2[:, 0:1], scalar2=None,
                                        op0=OP.mult)
                nc.vector.tensor_tensor(out=h2[:], in0=h2[:], in1=b2_sb[:],
                                        op=OP.add)
                nc.scalar.activation(h2[:], h2[:], ACT.Relu)
                nc.sync.dma_start(out=emb[b * P: b * P + nb, :], in_=h2[:nb, :])

                # logits = h2 @ fcW + fcb
                ptr2 = ps.tile([P, P], F32, tag="ptr", bufs=getattr(c, 'PTR_BUFS', 2))
                nc.tensor.transpose(ptr2[:c.C2, :], h2[:, :], ident_sb[:])
                h2t = sb.tile([c.C2, P], F32, tag="h2t")
                nc.vector.tensor_copy(out=h2t[:, :], in_=ptr2[:c.C2, :])
                plog = ps.tile([P, c.OUT], F32, tag="paux", bufs=getattr(c, 'PAUX_BUFS', 2))
                nc.tensor.matmul(plog[:], lhsT=h2t[:, :], rhs=fcW_sb[:],
                                 start=True, stop=True)
                lg = sb.tile([P, c.OUT], F32, tag="lg")
                nc.vector.tensor_tensor(out=lg[:], in0=plog[:], in1=fcb_sb[:],
                                        op=OP.add)
                nc.sync.dma_start(out=logits[b * P: b * P + nb, :],
                                  in_=lg[:nb, :])

    nc.finalize()
    lower_extended_insts(nc)
    _fix_multiwait(nc)
    return nc


# ---------------------------------------------------------------- inputs


def build_in_maps(inputs, prep, cfg):
    c = cfg
    x = np.asarray(inputs["x"], np.float32)
    W1 = np.ascontiguousarray(np.asarray(inputs["W1"], np.float32))
    att_src1 = np.asarray(inputs["att_src1"], np.float32).reshape(-1)
    att_dst1 = np.asarray(inputs["att_dst1"], np.float32).reshape(-1)
    b1 = np.asarray(inputs["b1"], np.float32).reshape(-1)
    W2 = np.ascontiguousarray(np.asarray(inputs["W2"], np.float32))
    att_src2 = np.asarray(inputs["att_src2"], np.float32).reshape(-1)
    att_dst2 = np.asarray(inputs["att_dst2"], np.float32).reshape(-1)
    b2 = np.asarray(inputs["b2"], np.float32).reshape(-1)
    fcW = np.ascontiguousarray(np.asarray(inputs["fcW"], np.float32))
    fcb = np.asarray(inputs["fcb"], np.float32).reshape(-1)

    brc = lambda v: np.ascontiguousarray(np.tile(v[None, :], (P, 1)))
    iota = np.tile(np.arange(P, dtype=np.float32), (P, 1))
    ident = np.eye(P, dtype=np.float32)

    shared = dict(W1=W1, at1s=brc(att_src1), at1d=brc(att_dst1), b1b=brc(b1),
                  W2=W2, at2s=brc(att_src2), at2d=brc(att_dst2), b2b=brc(b2),
                  fcW=fcW, fcbb=brc(fcb), iota=np.ascontiguousarray(iota),
                  ident=ident)
    in_maps = []
    for r in range(c.NC):
        pc = prep["cores"][r]
        xTo = np.ascontiguousarray(
            x[r * c.N_OWN:(r + 1) * c.N_OWN, :].T)
        in_maps.append(dict(shared, xT=xTo, lo_idx=pc["lo_idx"],
                            hi_idx=pc["hi_idx"], t_idx=pc["t_idx"],
                            dst_rel=pc["dst_rel"]))
    return in_maps


# ---------------------------------------------------------------- entry

_CACHE = {}


def _get_compiled(edge_index, cfg):
    key = hashlib.sha256(
        np.ascontiguousarray(np.asarray(edge_index)).tobytes()).hexdigest()
    hit = _CACHE.get(key)
    if hit is None:
        prep = preprocess(edge_index, cfg)
        nc = build_program(cfg, prep["T_lo"], prep["T_hi"])
        hit = (prep, nc)
        _CACHE[key] = hit
    return hit


def kernel(**inputs):
    cfg = Cfg(**FULL_CFG)
    prep, nc = _get_compiled(inputs["edge_index"], cfg)
    in_maps = build_in_maps(inputs, prep, cfg)

    from concourse.bass_utils import run_bass_kernel_spmd

    res = run_bass_kernel_spmd(nc, in_maps, core_ids=list(range(cfg.NC)))
    emb = np.concatenate([res.results[r]["emb"] for r in range(cfg.NC)])
    logits = np.concatenate([res.results[r]["logits"] for r in range(cfg.NC)])
    return (emb, logits)
